# revision 1
# baseline (speedup 1.0000x reference)
"""Trainium2 Bass kernel: 16-head attention with LoRA (B=2, N=2048, C=1024).

Sharding: batch x sequence rows across 8 cores (core c: batch c//4, rows
(c%4)*512). Heads stay whole per core; K/V are all-gathered over the 4-core
batch group. Everything on device is computed transposed (feature dim on
partitions), so no on-device transposes are needed; the host transposes the
per-core [1024, 512] output slabs back.
"""

import os
from contextlib import ExitStack

import numpy as np
import ml_dtypes

import concourse.bass as bass
import concourse.mybir as mybir
import concourse.tile as tile
from concourse.bass_utils import run_bass_kernel_spmd

B, N, C, H, D = 2, 2048, 1024, 16, 64
R = 512          # query rows per core
KT = N // 128    # 16 seq tiles of 128
BF = mybir.dt.bfloat16
F32 = mybir.dt.float32
GROUPS = [[0, 1, 2, 3], [4, 5, 6, 7]]


def build():
    nc = bass.Bass()
    xT = nc.declare_dram_parameter("xT", [C, R], BF, isOutput=False)
    wqkT = nc.declare_dram_parameter("wqkT", [C, 2 * C], BF, isOutput=False)
    wvT = nc.declare_dram_parameter("wvT", [C, C], BF, isOutput=False)
    a1T = nc.declare_dram_parameter("a1T", [C, 8], BF, isOutput=False)
    b1qkT = nc.declare_dram_parameter("b1qkT", [8, 2 * C], BF, isOutput=False)
    b1vT = nc.declare_dram_parameter("b1vT", [8, C], BF, isOutput=False)
    projT = nc.declare_dram_parameter("projT", [C, C], BF, isOutput=False)
    a2T = nc.declare_dram_parameter("a2T", [C, 8], BF, isOutput=False)
    b2aug = nc.declare_dram_parameter("b2aug", [9, C], BF, isOutput=False)
    outT = nc.declare_dram_parameter("outT", [C, R], F32, isOutput=True)

    with tile.TileContext(nc) as tc, ExitStack() as ctx:
        dram = ctx.enter_context(tc.tile_pool(name="dram", bufs=1, space="DRAM"))
        k_bounce = dram.tile([C, R], BF)
        v_bounce = dram.tile([R, C], BF)
        k_gath = dram.tile([4 * C, R], BF)
        v_gath = dram.tile([N, C], BF)

        cst = ctx.enter_context(tc.tile_pool(name="cst", bufs=1))
        xT_s = cst.tile([128, 8, R], BF)
        nc.sync.dma_start(out=xT_s, in_=xT[:, :].rearrange("(kt p) r -> p kt r", p=128))
        a1T_s = cst.tile([128, 8, 8], BF)
        nc.sync.dma_start(out=a1T_s, in_=a1T[:, :].rearrange("(kt p) e -> p kt e", p=128))
        wqkT_s = cst.tile([128, 8, 2 * C], BF)
        nc.sync.dma_start(out=wqkT_s, in_=wqkT[:, :].rearrange("(kt p) c -> p kt c", p=128))
        wvT_s = cst.tile([128, 8, C], BF)
        nc.sync.dma_start(out=wvT_s, in_=wvT[:, :].rearrange("(kt p) c -> p kt c", p=128))
        b1qkT_s = cst.tile([8, 2 * C], BF)
        nc.sync.dma_start(out=b1qkT_s, in_=b1qkT[:, :])
        b1vT_s = cst.tile([8, C], BF)
        nc.sync.dma_start(out=b1vT_s, in_=b1vT[:, :])
        projT_s = cst.tile([128, 8, C], BF)
        nc.sync.dma_start(out=projT_s, in_=projT[:, :].rearrange("(kt p) c -> p kt c", p=128))
        a2T_s = cst.tile([128, 8, 8], BF)
        nc.sync.dma_start(out=a2T_s, in_=a2T[:, :].rearrange("(kt p) e -> p kt e", p=128))
        b2aug_s = cst.tile([9, C], BF)
        nc.sync.dma_start(out=b2aug_s, in_=b2aug[:, :])

        xaT_s = cst.tile([8, R], BF)
        qkT_s = cst.tile([128, 16, R], BF)
        v_ls = cst.tile([128, 4, C], BF)
        att_s = cst.tile([128, 8, R], BF)

        # ---- phase 1: lora1 intermediate xaT = A1 @ x.T  [8, R]
        with tc.tile_pool(name="psA", bufs=1, space="PSUM") as psA:
            xa_ps = psA.tile([8, R], F32, tag="sm")
            for kt in range(8):
                nc.tensor.matmul(xa_ps, a1T_s[:, kt, :], xT_s[:, kt, :],
                                 start=(kt == 0), stop=(kt == 7))
            nc.vector.tensor_copy(xaT_s, xa_ps)

            # ---- phase 2: qkT = (qkv_w[:2C] @ x.T + lora), k-part first
            for ct in list(range(8, 16)) + list(range(8)):
                qk_ps = psA.tile([128, R], F32, tag="mm", bufs=2)
                for kt in range(8):
                    nc.tensor.matmul(qk_ps, wqkT_s[:, kt, ct * 128:(ct + 1) * 128],
                                     xT_s[:, kt, :], start=(kt == 0), stop=False)
                nc.tensor.matmul(qk_ps, b1qkT_s[:, ct * 128:(ct + 1) * 128], xaT_s,
                                 start=False, stop=True)
                nc.vector.tensor_copy(qkT_s[:, ct, :], qk_ps)
                if ct == 15:
                    nc.gpsimd.dma_start(
                        out=k_bounce[:, :].rearrange("(ct p) r -> p ct r", p=128),
                        in_=qkT_s[:, 8:16, :])
                    nc.gpsimd.collective_compute(
                        "AllGather", mybir.AluOpType.bypass,
                        ins=[k_bounce.opt()], outs=[k_gath.opt()],
                        replica_groups=GROUPS)

            # ---- phase 3: v natural = x @ wv.T + lora   [R, C]
            for rt in range(4):
                for vct in range(2):
                    v_ps = psA.tile([128, 512], F32, tag="mm", bufs=2)
                    for kt in range(8):
                        nc.tensor.matmul(v_ps, xT_s[:, kt, rt * 128:(rt + 1) * 128],
                                         wvT_s[:, kt, vct * 512:(vct + 1) * 512],
                                         start=(kt == 0), stop=False)
                    nc.tensor.matmul(v_ps, xaT_s[:, rt * 128:(rt + 1) * 128],
                                     b1vT_s[:, vct * 512:(vct + 1) * 512],
                                     start=False, stop=True)
                    nc.vector.tensor_copy(v_ls[:, rt, vct * 512:(vct + 1) * 512], v_ps)
            nc.gpsimd.dma_start(
                out=v_bounce[:, :].rearrange("(rt p) c -> p rt c", p=128),
                in_=v_ls)
            nc.gpsimd.collective_compute(
                "AllGather", mybir.AluOpType.bypass,
                ins=[v_bounce.opt()], outs=[v_gath.opt()],
                replica_groups=GROUPS)

        v_gr = v_gath[:, :].rearrange("(kt p) c -> p kt c", p=128)

        # ---- phase 4+5 pools open together (no sem-frontier collapse at
        # a PSUM pool boundary; walrus caps sync waits per instruction)
        with tc.tile_pool(name="psB", bufs=1, space="PSUM") as psB, \
             tc.tile_pool(name="psC", bufs=1, space="PSUM") as psC, \
             tc.tile_pool(name="atn", bufs=1) as atn, \
             tc.tile_pool(name="prj", bufs=1) as prj:
            oa_aug = prj.tile([9, R], BF)
            nc.vector.memset(oa_aug, 1.0)
            for kp in range(8):
                kT_p = atn.tile([128, N], BF, tag="ktp", bufs=2)
                kg = k_gath[:, :]
                nc.sync.dma_start(
                    out=kT_p[:, :].rearrange("p (rk r) -> p rk r", rk=4),
                    in_=bass.AP(tensor=kg.tensor,
                                offset=kg.offset + kp * 128 * R,
                                ap=[[R, 128], [C * R, 4], [1, R]]))
                vs = []
                exps = []
                att_os = []
                for j in range(2):
                    h = 2 * kp + j
                    v_sj = atn.tile([128, KT, 65], BF, tag="vs", bufs=4)
                    nc.vector.memset(v_sj, 1.0)
                    nc.gpsimd.dma_start(out=v_sj[:, :, 0:64],
                                        in_=v_gr[:, :, h * 64:(h + 1) * 64])
                    vs.append(v_sj)
                    exps.append(atn.tile([128, KT, R], BF, tag="exps", bufs=2,
                                         name=f"exp_{kp}_{j}"))
                    att_os.append(psB.tile([65, R], F32, tag="ao", bufs=2,
                                           name=f"ao_{kp}_{j}"))
                # scores + exp, heads interleaved so K=64 row-groups pack
                for g in range(8):
                    sp = [psB.tile([128, 2, R], F32, tag="sc", bufs=2,
                                   name=f"sc_{kp}_{g}_{j}") for j in range(2)]
                    for jj in range(2):
                        kt = 2 * g + jj
                        for j in range(2):
                            nc.tensor.matmul(
                                sp[j][:, jj, :],
                                kT_p[j * 64:(j + 1) * 64, kt * 128:(kt + 1) * 128],
                                qkT_s[j * 64:(j + 1) * 64, kp, :],
                                start=True, stop=True)
                    for j in range(2):
                        nc.scalar.activation(exps[j][:, 2 * g:2 * g + 2, :], sp[j],
                                             mybir.ActivationFunctionType.Exp,
                                             scale=0.125)
                # attn @ V (transposed), with ones column giving the softmax denom
                for j in range(2):
                    for kt in range(KT):
                        nc.tensor.matmul(att_os[j], vs[j][:, kt, :], exps[j][:, kt, :],
                                         start=(kt == 0), stop=(kt == KT - 1))
                for j in range(2):
                    rr = atn.tile([65, R], F32, tag="rr", bufs=2)
                    nc.vector.reciprocal(rr[64:65, :], att_os[j][64:65, :])
                    rr_d = dram.tile([1, R], F32, tag="rrd", bufs=4,
                                     name=f"rrd_{kp}_{j}")
                    nc.gpsimd.dma_start(out=rr_d, in_=rr[64:65, :])
                    rd = rr_d[:, :]
                    rb = atn.tile([64, R], F32, tag="rb", bufs=2)
                    nc.sync.dma_start(out=rb, in_=bass.AP(
                        tensor=rd.tensor, offset=rd.offset,
                        ap=[[0, 64]] + [list(d) for d in rd.ap[1:]]))
                    if j == 0:
                        nc.vector.tensor_mul(att_s[0:64, kp, :], att_os[j][0:64, :], rb)
                    else:
                        tmp = atn.tile([64, R], BF, tag="atmp", bufs=2)
                        nc.vector.tensor_mul(tmp, att_os[j][0:64, :], rb)
                        nc.gpsimd.dma_start(out=att_s[64:128, kp, :], in_=tmp)

            # ---- phase 5: output projection with lora2 + bias
            oa_ps = psC.tile([8, R], F32, tag="sm")
            for kp in range(8):
                nc.tensor.matmul(oa_ps, a2T_s[:, kp, :], att_s[:, kp, :],
                                 start=(kp == 0), stop=(kp == 7))
            nc.vector.tensor_copy(oa_aug[0:8, :], oa_ps)
            for ct in range(8):
                f_ps = psC.tile([128, R], F32, tag="fm", bufs=1)
                for kp in range(8):
                    nc.tensor.matmul(f_ps, projT_s[:, kp, ct * 128:(ct + 1) * 128],
                                     att_s[:, kp, :], start=(kp == 0), stop=False)
                nc.tensor.matmul(f_ps, b2aug_s[:, ct * 128:(ct + 1) * 128], oa_aug,
                                 start=False, stop=True)
                f_s = prj.tile([128, R], F32, tag="fs", bufs=2)
                nc.vector.tensor_copy(f_s, f_ps)
                nc.gpsimd.dma_start(out=outT[ct * 128:(ct + 1) * 128, :], in_=f_s)
    _split_multi_waits(nc)
    return nc


def _split_multi_waits(nc):
    """This container's walrus supports one sync-wait per instruction; move
    extra waits onto preceding same-engine NoOps."""
    n_new = 0
    for bb in nc.m.functions[0].blocks:
        new = []
        for ins in bb.instructions:
            si = getattr(ins, "sync_info", None)
            ow = list(si.on_wait) if si is not None and si.on_wait else []
            if len(ow) > 1:
                for w in ow[:-1]:
                    n_new += 1
                    nop = mybir.InstNoOp(
                        name=f"{ins.name}_sw{n_new}",
                        engine=ins.engine,
                        sync_info=mybir.SyncInfo(on_wait=[w], on_update=[]),
                    )
                    new.append(nop)
                ins.sync_info = mybir.SyncInfo(
                    on_wait=[ow[-1]],
                    on_update=list(si.on_update) if si.on_update else [],
                )
            new.append(ins)
        bb.instructions = new


_NC = None
_LAST = None


def _ensure_ntff_hook():
    """The agent image's antenv lacks axon_hooks; shim it and register the
    ctypes NTFF profiler from trn_boot so trace=True yields exec_time_ns."""
    import sys
    import types
    try:
        import antenv.axon_hooks  # noqa: F401
        return
    except ImportError:
        pass
    mod = types.ModuleType("antenv.axon_hooks")
    holder = [None]
    mod.set_axon_ntff_profile_hook = lambda h: holder.__setitem__(0, h)
    mod.get_axon_ntff_profile_hook = lambda: holder[0]
    sys.modules["antenv.axon_hooks"] = mod
    import antenv
    antenv.axon_hooks = mod
    try:
        sys.path.insert(0, "/root/.axon_site")
        from trn_agent_boot.trn_boot import _ntff_profile_via_ctypes
        mod.set_axon_ntff_profile_hook(
            _ntff_profile_via_ctypes("/opt/axon/libaxon_pjrt.so"))
    except Exception:
        pass


def kernel(**inputs):
    global _NC, _LAST
    bf = ml_dtypes.bfloat16
    x = np.asarray(inputs["x"], np.float32)
    qkv_w = np.asarray(inputs["qkv_w"], np.float32)
    proj_w = np.asarray(inputs["proj_w"], np.float32)
    proj_b = np.asarray(inputs["proj_b"], np.float32)
    a1 = np.asarray(inputs["lora_w1_l1"], np.float32)
    b1 = np.asarray(inputs["lora_w1_l2"], np.float32)
    a2 = np.asarray(inputs["lora_w2_l1"], np.float32)
    b2 = np.asarray(inputs["lora_w2_l2"], np.float32)

    shared = {
        "wqkT": np.ascontiguousarray(qkv_w[:2 * C].T).astype(bf),
        "wvT": np.ascontiguousarray(qkv_w[2 * C:].T).astype(bf),
        "a1T": np.ascontiguousarray(a1.T).astype(bf),
        "b1qkT": np.ascontiguousarray(b1[:2 * C].T * 2.0).astype(bf),
        "b1vT": np.ascontiguousarray(b1[2 * C:].T * 2.0).astype(bf),
        "projT": np.ascontiguousarray(proj_w.T).astype(bf),
        "a2T": np.ascontiguousarray(a2.T).astype(bf),
        "b2aug": np.ascontiguousarray(
            np.vstack([b2.T * 2.0, proj_b[None, :]])).astype(bf),
    }
    in_maps = []
    for c in range(8):
        g, r = divmod(c, 4)
        m = dict(shared)
        m["xT"] = np.ascontiguousarray(x[g, r * R:(r + 1) * R, :].T).astype(bf)
        in_maps.append(m)

    if _NC is None:
        _NC = build()
    trace = os.environ.get("ATT_TRACE", "0") == "1"
    if trace:
        _ensure_ntff_hook()
    _LAST = run_bass_kernel_spmd(_NC, in_maps, core_ids=list(range(8)),
                                 trace=trace)
    out = np.empty((B, N, C), np.float32)
    for c in range(8):
        g, r = divmod(c, 4)
        out[g, r * R:(r + 1) * R, :] = np.asarray(
            _LAST.results[c]["outT"], np.float32).T
    return out



# revision 15
# speedup vs baseline: 1.2336x; 1.2336x over previous
"""Trainium2 Bass kernel: 16-head attention with LoRA (B=2, N=2048, C=1024).

v2: head-group sharding, no collectives. Core c handles batch c//4 and the
4 heads [4*(c%4), 4*(c%4)+4) over the FULL 2048-row sequence. LoRA is folded
into the qkv/proj weights on the host (W' = W + 2*B@A, exact). Each core
computes q,k,v for its heads, runs attention, and emits the partial output
projection over its 256 local channels; the host sums the 4 partials per
batch and adds the bias.

Pipeline: per (query-chunk, head-pair) block, scores (PE, row-packed K=64
pairs) feed exp (Scalar) feed attnV (PE, interleaved one block behind), so
ScalarE's exp stream and the PE matmul stream overlap continuously.
"""

import os
from contextlib import ExitStack

import numpy as np
import ml_dtypes

import concourse.bass as bass
import concourse.mybir as mybir
import concourse.tile as tile
from concourse.bass_utils import run_bass_kernel_spmd

B, N, C, H, D = 2, 2048, 1024, 16, 64
KT = N // 128    # 16 kv tiles of 128
QC = 4           # query chunks of 512
BF = mybir.dt.bfloat16
F32 = mybir.dt.float32
BLOCKS = [(qc, hp) for qc in range(QC) for hp in range(2)]


def build():
    nc = bass.Bass()
    xT = nc.declare_dram_parameter("xT", [C, N], BF, isOutput=False)
    wqkT = nc.declare_dram_parameter("wqkT", [C, 512], BF, isOutput=False)
    wvT = nc.declare_dram_parameter("wvT", [C, 256], BF, isOutput=False)
    projT = nc.declare_dram_parameter("projT", [256, C], BF, isOutput=False)
    outT = nc.declare_dram_parameter("outT", [C, N], F32, isOutput=True)

    with tile.TileContext(nc) as tc, ExitStack() as ctx:
        dram = ctx.enter_context(tc.tile_pool(name="dram", bufs=1, space="DRAM"))
        cst = ctx.enter_context(tc.tile_pool(name="cst", bufs=1))
        atn = ctx.enter_context(tc.tile_pool(name="atn", bufs=1))
        ps = ctx.enter_context(tc.tile_pool(name="ps", bufs=1, space="PSUM"))

        # ---- persistent SBUF tiles
        wqkT_s = cst.tile([128, 8, 512], BF)
        nc.gpsimd.dma_start(out=wqkT_s, in_=wqkT[:, :].rearrange("(kt p) c -> p kt c", p=128))
        xT_s = cst.tile([128, 8, N], BF)
        for qc in range(QC):
            nc.sync.dma_start(
                out=xT_s[:, :, qc * 512:(qc + 1) * 512],
                in_=xT[:, qc * 512:(qc + 1) * 512].rearrange("(kt p) n -> p kt n", p=128))
        wvT_s = cst.tile([128, 8, 256], BF)
        nc.gpsimd.dma_start(out=wvT_s, in_=wvT[:, :].rearrange("(kt p) c -> p kt c", p=128))
        projT_s = cst.tile([128, 2, C], BF)
        nc.gpsimd.dma_start(out=projT_s, in_=projT[:, :].rearrange("(hp p) c -> p hp c", p=128))

        kT_s = cst.tile([128, 2, N], BF)       # K^T per head pair
        qT_s = cst.tile([128, 2, N], BF)       # Q^T per head pair
        v_s = cst.tile([128, KT, 260], BF)     # V per head (4x65 blocks, col 64 = ones)
        nc.vector.memset(v_s, 1.0)
        attn_s = cst.tile([128, 2, N], BF)     # normalized O^T per pair

        exps = {}
        aos = {}

        def qk_chunk(dst, col, qc, nm):
            p_ps = ps.tile([128, 512], F32, tag="sc", bufs=2, name=f"qk_{nm}")
            for kt in range(8):
                nc.tensor.matmul(p_ps, wqkT_s[:, kt, col * 128:(col + 1) * 128],
                                 xT_s[:, kt, qc * 512:(qc + 1) * 512],
                                 start=(kt == 0), stop=(kt == 7))
            nc.vector.tensor_copy(dst, p_ps)

        def scores_g(b, g):
            qc, hp = BLOCKS[b]
            if g == 0:
                exps[b] = [atn.tile([128, KT, 512], BF, tag=f"exps{j}", bufs=2,
                                    name=f"exps{j}_{b}") for j in range(2)]
            sp = [ps.tile([128, 2, 512], F32, tag="sc", bufs=2,
                          name=f"sc_{b}_{g}_{j}") for j in range(2)]
            for jj in range(2):
                kt = 2 * g + jj
                for j in range(2):
                    nc.tensor.matmul(
                        sp[j][:, jj, :],
                        kT_s[j * 64:(j + 1) * 64, hp, kt * 128:(kt + 1) * 128],
                        qT_s[j * 64:(j + 1) * 64, hp, qc * 512:(qc + 1) * 512],
                        start=True, stop=True)
            for j in range(2):
                nc.scalar.activation(exps[b][j][:, 2 * g:2 * g + 2, :], sp[j],
                                     mybir.ActivationFunctionType.Exp, scale=0.125)

        def av_g(b, g):
            qc, hp = BLOCKS[b]
            if g == 0:
                aos[b] = [ps.tile([65, 512], F32, tag=f"ao{j}", bufs=2,
                                  name=f"ao_{b}_{j}") for j in range(2)]
            for jj in range(2):
                kt = 2 * g + jj
                for j in range(2):
                    h = 2 * hp + j
                    nc.tensor.matmul(aos[b][j], v_s[:, kt, h * 65:h * 65 + 65],
                                     exps[b][j][:, kt, :],
                                     start=(kt == 0), stop=(kt == KT - 1))

        def norm(b):
            qc, hp = BLOCKS[b]
            for j in range(2):
                ao = aos[b][j]
                # denominator -> DRAM -> [128,4] -> reciprocal -> DRAM -> [64,512] bcast
                den_s = atn.tile([1, 512], F32, tag="dens", bufs=2, name=f"den_{b}_{j}")
                nc.vector.tensor_copy(den_s, ao[64:65, :])
                dd = dram.tile([1, 512], F32, tag="rrd", bufs=4, name=f"dd_{b}_{j}")
                nc.gpsimd.dma_start(out=dd, in_=den_s)
                dt = atn.tile([128, 4], F32, tag="dt", bufs=2, name=f"dt_{b}_{j}")
                ddp = dd[:, :]
                nc.sync.dma_start(out=dt, in_=bass.AP(
                    tensor=ddp.tensor, offset=ddp.offset, ap=[[1, 128], [128, 4]]))
                rt = atn.tile([128, 4], F32, tag="rt", bufs=2, name=f"rt_{b}_{j}")
                nc.vector.reciprocal(rt, dt)
                rd = dram.tile([1, 512], F32, tag="rtd", bufs=4, name=f"rd_{b}_{j}")
                rdp = rd[:, :]
                # store transposed: rt[p,k] -> rd[k*128+p], so rd is linear in q
                nc.gpsimd.dma_start(
                    out=bass.AP(tensor=rdp.tensor, offset=rdp.offset,
                                ap=[[1, 128], [128, 4]]),
                    in_=rt)
                rb = atn.tile([64, 512], F32, tag="rb", bufs=2, name=f"rb_{b}_{j}")
                nc.sync.dma_start(out=rb, in_=bass.AP(
                    tensor=rdp.tensor, offset=rdp.offset,
                    ap=[[0, 64], [1, 512]]))
                if j == 0:
                    nc.vector.tensor_mul(attn_s[0:64, hp, qc * 512:(qc + 1) * 512],
                                         ao[0:64, :], rb)
                else:
                    tmp = atn.tile([64, 512], BF, tag="atmp", bufs=2, name=f"tmp_{b}")
                    nc.vector.tensor_mul(tmp, ao[0:64, :], rb)
                    nc.gpsimd.dma_start(
                        out=attn_s[64:128, hp, qc * 512:(qc + 1) * 512], in_=tmp)

        def proj(qc):
            for ct in range(8):
                f_ps = ps.tile([128, 512], F32, tag="sc", bufs=2, name=f"f_{qc}_{ct}")
                for hp in range(2):
                    nc.tensor.matmul(f_ps, projT_s[:, hp, ct * 128:(ct + 1) * 128],
                                     attn_s[:, hp, qc * 512:(qc + 1) * 512],
                                     start=(hp == 0), stop=(hp == 1))
                f_s = atn.tile([128, 512], F32, tag="fs", bufs=4, name=f"fs_{qc}_{ct}")
                nc.vector.tensor_copy(f_s, f_ps)
                nc.gpsimd.dma_start(
                    out=outT[ct * 128:(ct + 1) * 128, qc * 512:(qc + 1) * 512], in_=f_s)

        # ---- phase B interleaved with first two score blocks
        for qc in range(QC):
            qk_chunk(kT_s[:, 0, qc * 512:(qc + 1) * 512], 0, qc, f"k0{qc}")
        qk_chunk(qT_s[:, 0, 0:512], 2, 0, "q00")
        for g in range(8):
            scores_g(0, g)
        for qc in range(QC):
            qk_chunk(kT_s[:, 1, qc * 512:(qc + 1) * 512], 1, qc, f"k1{qc}")
        qk_chunk(qT_s[:, 1, 0:512], 3, 0, "q10")
        for g in range(8):
            scores_g(1, g)

        # ---- V (16 row tiles) with av(block 0) interleaved
        for rt in range(KT):
            v_ps = ps.tile([128, 256], F32, tag="sc", bufs=2, name=f"v_{rt}")
            for kt in range(8):
                nc.tensor.matmul(v_ps, xT_s[:, kt, rt * 128:(rt + 1) * 128],
                                 wvT_s[:, kt, :], start=(kt == 0), stop=(kt == 7))
            vr = v_s[:, rt, 0:260]
            dst = bass.AP(tensor=vr.tensor, offset=vr.offset,
                          ap=[list(vr.ap[0]), [65, 4], [1, 64]])
            nc.vector.tensor_copy(dst, v_ps[:, :].rearrange("p (h e) -> p h e", h=4))
            if rt >= 8:
                av_g(0, rt - 8)
        norm(0)

        # ---- main loop: scores(b) + av(b-1) interleaved; Q chunks staggered
        for b in range(2, len(BLOCKS)):
            if b % 2 == 0:
                qc = b // 2
                qk_chunk(qT_s[:, 0, qc * 512:(qc + 1) * 512], 2, qc, f"q0{qc}")
                qk_chunk(qT_s[:, 1, qc * 512:(qc + 1) * 512], 3, qc, f"q1{qc}")
            for g in range(8):
                scores_g(b, g)
                av_g(b - 1, g)
            norm(b - 1)
            pqc, php = BLOCKS[b - 1]
            if php == 1:
                proj(pqc)
        # ---- tail: av + norm of last block, final projection chunk
        for g in range(8):
            av_g(len(BLOCKS) - 1, g)
        norm(len(BLOCKS) - 1)
        proj(QC - 1)
    _split_multi_waits(nc)
    return nc


def _split_multi_waits(nc):
    """This container's walrus supports one sync-wait per instruction; move
    extra waits onto preceding same-engine NoOps."""
    n_new = 0
    for bb in nc.m.functions[0].blocks:
        new = []
        for ins in bb.instructions:
            si = getattr(ins, "sync_info", None)
            ow = list(si.on_wait) if si is not None and si.on_wait else []
            if len(ow) > 1:
                for w in ow[:-1]:
                    n_new += 1
                    nop = mybir.InstNoOp(
                        name=f"{ins.name}_sw{n_new}",
                        engine=ins.engine,
                        sync_info=mybir.SyncInfo(on_wait=[w], on_update=[]),
                    )
                    new.append(nop)
                ins.sync_info = mybir.SyncInfo(
                    on_wait=[ow[-1]],
                    on_update=list(si.on_update) if si.on_update else [],
                )
            new.append(ins)
        bb.instructions = new


_NC = None
_LAST = None


def _ensure_ntff_hook():
    """The agent image's antenv lacks axon_hooks; shim it and register the
    ctypes NTFF profiler from trn_boot so trace=True yields exec_time_ns."""
    import sys
    import types
    try:
        import antenv.axon_hooks  # noqa: F401
        return
    except ImportError:
        pass
    mod = types.ModuleType("antenv.axon_hooks")
    holder = [None]
    mod.set_axon_ntff_profile_hook = lambda h: holder.__setitem__(0, h)
    mod.get_axon_ntff_profile_hook = lambda: holder[0]
    sys.modules["antenv.axon_hooks"] = mod
    import antenv
    antenv.axon_hooks = mod
    try:
        sys.path.insert(0, "/root/.axon_site")
        from trn_agent_boot.trn_boot import _ntff_profile_via_ctypes
        mod.set_axon_ntff_profile_hook(
            _ntff_profile_via_ctypes("/opt/axon/libaxon_pjrt.so"))
    except Exception:
        pass


def kernel(**inputs):
    global _NC, _LAST
    bf = ml_dtypes.bfloat16
    x = np.asarray(inputs["x"], np.float32)
    qkv_w = np.asarray(inputs["qkv_w"], np.float32)
    proj_w = np.asarray(inputs["proj_w"], np.float32)
    proj_b = np.asarray(inputs["proj_b"], np.float32)
    a1 = np.asarray(inputs["lora_w1_l1"], np.float32)
    b1 = np.asarray(inputs["lora_w1_l2"], np.float32)
    a2 = np.asarray(inputs["lora_w2_l1"], np.float32)
    b2 = np.asarray(inputs["lora_w2_l2"], np.float32)

    # fold LoRA into the dense weights (exact: x@W.T + (x@A.T)@B.T*2 = x@(W+2BA).T)
    Wqkv = qkv_w + 2.0 * (b1 @ a1)
    Wp = proj_w + 2.0 * (b2 @ a2)

    xTg = [np.ascontiguousarray(x[g].T).astype(bf) for g in range(B)]
    in_maps = []
    for c in range(8):
        g, hg = divmod(c, 4)
        r0 = hg * 256
        Kg = Wqkv[1024 + r0:1024 + r0 + 256]        # [256, 1024]
        Qg = Wqkv[r0:r0 + 256]
        Vg = Wqkv[2048 + r0:2048 + r0 + 256]
        m = {
            "xT": xTg[g],
            "wqkT": np.ascontiguousarray(np.vstack([Kg, Qg]).T).astype(bf),
            "wvT": np.ascontiguousarray(Vg.T).astype(bf),
            "projT": np.ascontiguousarray(Wp[:, r0:r0 + 256].T).astype(bf),
        }
        in_maps.append(m)

    if _NC is None:
        _NC = build()
    trace = os.environ.get("ATT_TRACE", "0") == "1"
    if trace:
        _ensure_ntff_hook()
    _LAST = run_bass_kernel_spmd(_NC, in_maps, core_ids=list(range(8)),
                                 trace=trace)
    out = np.empty((B, N, C), np.float32)
    for g in range(B):
        acc = np.zeros((C, N), np.float32)
        for hg in range(4):
            acc += np.asarray(_LAST.results[4 * g + hg]["outT"], np.float32)
        out[g] = acc.T + proj_b[None, :]
    return out


# revision 16
# speedup vs baseline: 1.2816x; 1.0389x over previous
"""Trainium2 Bass kernel: 16-head attention with LoRA (B=2, N=2048, C=1024).

v2: head-group sharding, no collectives. Core c handles batch c//4 and the
4 heads [4*(c%4), 4*(c%4)+4) over the FULL 2048-row sequence. LoRA is folded
into the qkv/proj weights on the host (W' = W + 2*B@A, exact). Each core
computes q,k,v for its heads, runs attention, and emits the partial output
projection over its 256 local channels; the host sums the 4 partials per
batch and adds the bias.

Pipeline: per (query-chunk, head-pair) block, scores (PE, row-packed K=64
pairs) feed exp (Scalar) feed attnV (PE, interleaved one block behind), so
ScalarE's exp stream and the PE matmul stream overlap continuously.
"""

import os
from contextlib import ExitStack

import numpy as np
import ml_dtypes

import concourse.bass as bass
import concourse.mybir as mybir
import concourse.tile as tile
from concourse.bass_utils import run_bass_kernel_spmd

B, N, C, H, D = 2, 2048, 1024, 16, 64
KT = N // 128    # 16 kv tiles of 128
QC = 4           # query chunks of 512
BF = mybir.dt.bfloat16
F32 = mybir.dt.float32
BLOCKS = [(qc, hp) for qc in range(QC) for hp in range(2)]


def build():
    nc = bass.Bass()
    xT = nc.declare_dram_parameter("xT", [C, N], BF, isOutput=False)
    wqkT = nc.declare_dram_parameter("wqkT", [C, 512], BF, isOutput=False)
    wvT = nc.declare_dram_parameter("wvT", [C, 256], BF, isOutput=False)
    projT = nc.declare_dram_parameter("projT", [256, C], BF, isOutput=False)
    outT = nc.declare_dram_parameter("outT", [C, N], F32, isOutput=True)

    with tile.TileContext(nc) as tc, ExitStack() as ctx:
        dram = ctx.enter_context(tc.tile_pool(name="dram", bufs=1, space="DRAM"))
        cst = ctx.enter_context(tc.tile_pool(name="cst", bufs=1))
        atn = ctx.enter_context(tc.tile_pool(name="atn", bufs=1))
        ps = ctx.enter_context(tc.tile_pool(name="ps", bufs=1, space="PSUM"))

        # ---- persistent SBUF tiles
        wqkT_s = cst.tile([128, 8, 512], BF)
        nc.gpsimd.dma_start(out=wqkT_s, in_=wqkT[:, :].rearrange("(kt p) c -> p kt c", p=128))
        xT_s = cst.tile([128, 8, N], BF)
        for qc in range(QC):
            nc.sync.dma_start(
                out=xT_s[:, :, qc * 512:(qc + 1) * 512],
                in_=xT[:, qc * 512:(qc + 1) * 512].rearrange("(kt p) n -> p kt n", p=128))
        wvT_s = cst.tile([128, 8, 256], BF)
        nc.gpsimd.dma_start(out=wvT_s, in_=wvT[:, :].rearrange("(kt p) c -> p kt c", p=128))
        projT_s = cst.tile([128, 2, C], BF)
        nc.gpsimd.dma_start(out=projT_s, in_=projT[:, :].rearrange("(hp p) c -> p hp c", p=128))

        kT_s = cst.tile([128, 2, N], BF)       # K^T per head pair
        qT_s = cst.tile([128, 2, N], BF)       # Q^T per head pair
        v_s = cst.tile([128, KT, 260], BF)     # V per head (4x65 blocks, col 64 = ones)
        nc.vector.memset(v_s, 1.0)
        attn_s = cst.tile([128, 2, N], BF)     # normalized O^T per pair

        exps = {}
        aos = {}

        def qk_chunk(dst, col, qc, nm):
            p_ps = ps.tile([128, 512], F32, tag="sc", bufs=2, name=f"qk_{nm}")
            for kt in range(8):
                nc.tensor.matmul(p_ps, wqkT_s[:, kt, col * 128:(col + 1) * 128],
                                 xT_s[:, kt, qc * 512:(qc + 1) * 512],
                                 start=(kt == 0), stop=(kt == 7))
            nc.vector.tensor_copy(dst, p_ps)

        def scores_g(b, g):
            qc, hp = BLOCKS[b]
            if g == 0:
                exps[b] = [atn.tile([128, KT, 512], BF, tag=f"exps{j}", bufs=2,
                                    name=f"exps{j}_{b}") for j in range(2)]
            sp = [ps.tile([128, 2, 512], F32, tag="sc", bufs=2,
                          name=f"sc_{b}_{g}_{j}") for j in range(2)]
            for jj in range(2):
                kt = 2 * g + jj
                for j in range(2):
                    nc.tensor.matmul(
                        sp[j][:, jj, :],
                        kT_s[j * 64:(j + 1) * 64, hp, kt * 128:(kt + 1) * 128],
                        qT_s[j * 64:(j + 1) * 64, hp, qc * 512:(qc + 1) * 512],
                        start=True, stop=True)
            for j in range(2):
                nc.scalar.activation(exps[b][j][:, 2 * g:2 * g + 2, :], sp[j],
                                     mybir.ActivationFunctionType.Exp, scale=0.125)

        def av_g(b, g):
            qc, hp = BLOCKS[b]
            if g == 0:
                aos[b] = [ps.tile([65, 512], F32, tag=f"ao{j}", bufs=2,
                                  name=f"ao_{b}_{j}") for j in range(2)]
            for jj in range(2):
                kt = 2 * g + jj
                for j in range(2):
                    h = 2 * hp + j
                    nc.tensor.matmul(aos[b][j], v_s[:, kt, h * 65:h * 65 + 65],
                                     exps[b][j][:, kt, :],
                                     start=(kt == 0), stop=(kt == KT - 1))

        def norm(b):
            qc, hp = BLOCKS[b]
            for j in range(2):
                ao = aos[b][j]
                # denominator -> DRAM -> [128,4] -> reciprocal -> DRAM -> [64,512] bcast
                den_s = atn.tile([1, 512], F32, tag="dens", bufs=2, name=f"den_{b}_{j}")
                nc.vector.tensor_copy(den_s, ao[64:65, :])
                dd = dram.tile([1, 512], F32, tag="rrd", bufs=4, name=f"dd_{b}_{j}")
                nc.gpsimd.dma_start(out=dd, in_=den_s)
                dt = atn.tile([128, 4], F32, tag="dt", bufs=2, name=f"dt_{b}_{j}")
                ddp = dd[:, :]
                nc.sync.dma_start(out=dt, in_=bass.AP(
                    tensor=ddp.tensor, offset=ddp.offset, ap=[[1, 128], [128, 4]]))
                rt = atn.tile([128, 4], F32, tag="rt", bufs=2, name=f"rt_{b}_{j}")
                nc.vector.reciprocal(rt, dt)
                rd = dram.tile([1, 512], F32, tag="rtd", bufs=4, name=f"rd_{b}_{j}")
                rdp = rd[:, :]
                # store transposed: rt[p,k] -> rd[k*128+p], so rd is linear in q
                nc.gpsimd.dma_start(
                    out=bass.AP(tensor=rdp.tensor, offset=rdp.offset,
                                ap=[[1, 128], [128, 4]]),
                    in_=rt)
                rb = atn.tile([64, 512], F32, tag="rb", bufs=2, name=f"rb_{b}_{j}")
                nc.sync.dma_start(out=rb, in_=bass.AP(
                    tensor=rdp.tensor, offset=rdp.offset,
                    ap=[[0, 64], [1, 512]]))
                if j == 0:
                    nc.vector.tensor_mul(attn_s[0:64, hp, qc * 512:(qc + 1) * 512],
                                         ao[0:64, :], rb)
                else:
                    tmp = atn.tile([64, 512], BF, tag="atmp", bufs=2, name=f"tmp_{b}")
                    nc.vector.tensor_mul(tmp, ao[0:64, :], rb)
                    nc.gpsimd.dma_start(
                        out=attn_s[64:128, hp, qc * 512:(qc + 1) * 512], in_=tmp)

        def proj(qc):
            for ct in range(8):
                f_ps = ps.tile([128, 512], F32, tag="sc", bufs=2, name=f"f_{qc}_{ct}")
                for hp in range(2):
                    nc.tensor.matmul(f_ps, projT_s[:, hp, ct * 128:(ct + 1) * 128],
                                     attn_s[:, hp, qc * 512:(qc + 1) * 512],
                                     start=(hp == 0), stop=(hp == 1))
                f_s = atn.tile([128, 512], F32, tag="fs", bufs=4, name=f"fs_{qc}_{ct}")
                nc.vector.tensor_copy(f_s, f_ps)
                nc.gpsimd.dma_start(
                    out=outT[ct * 128:(ct + 1) * 128, qc * 512:(qc + 1) * 512], in_=f_s)

        # ---- phase B interleaved with first two score blocks
        for qc in range(QC):
            qk_chunk(kT_s[:, 0, qc * 512:(qc + 1) * 512], 0, qc, f"k0{qc}")
        qk_chunk(qT_s[:, 0, 0:512], 2, 0, "q00")
        for g in range(8):
            scores_g(0, g)
        for qc in range(QC):
            qk_chunk(kT_s[:, 1, qc * 512:(qc + 1) * 512], 1, qc, f"k1{qc}")
        qk_chunk(qT_s[:, 1, 0:512], 3, 0, "q10")
        for g in range(8):
            scores_g(1, g)

        # ---- V (16 row tiles) with av(block 0) and Q(qc1) chunks interleaved
        for rt in range(KT):
            v_ps = ps.tile([128, 256], F32, tag="sc", bufs=2, name=f"v_{rt}")
            for kt in range(8):
                nc.tensor.matmul(v_ps, xT_s[:, kt, rt * 128:(rt + 1) * 128],
                                 wvT_s[:, kt, :], start=(kt == 0), stop=(kt == 7))
            vr = v_s[:, rt, 0:260]
            dst = bass.AP(tensor=vr.tensor, offset=vr.offset,
                          ap=[list(vr.ap[0]), [65, 4], [1, 64]])
            nc.vector.tensor_copy(dst, v_ps[:, :].rearrange("p (h e) -> p h e", h=4))
            if rt >= 8:
                av_g(0, rt - 8)
            if rt == 12:
                qk_chunk(qT_s[:, 0, 512:1024], 2, 1, "q01")
            if rt == 14:
                qk_chunk(qT_s[:, 1, 512:1024], 3, 1, "q11")
        norm(0)

        # ---- main loop: scores(b) + av(b-1) interleaved; Q chunks for qc+1
        # injected mid-g-loop of the preceding odd block (ring slot is free then)
        for b in range(2, len(BLOCKS)):
            for g in range(8):
                scores_g(b, g)
                av_g(b - 1, g)
                if b in (3, 5) and g in (4, 6):
                    nqc = (b + 1) // 2
                    hp = 0 if g == 4 else 1
                    qk_chunk(qT_s[:, hp, nqc * 512:(nqc + 1) * 512],
                             2 + hp, nqc, f"q{hp}{nqc}")
            norm(b - 1)
            pqc, php = BLOCKS[b - 1]
            if php == 1:
                proj(pqc)
        # ---- tail: av + norm of last block, final projection chunk
        for g in range(8):
            av_g(len(BLOCKS) - 1, g)
        norm(len(BLOCKS) - 1)
        proj(QC - 1)
    _split_multi_waits(nc)
    return nc


def _split_multi_waits(nc):
    """This container's walrus supports one sync-wait per instruction; move
    extra waits onto preceding same-engine NoOps."""
    n_new = 0
    for bb in nc.m.functions[0].blocks:
        new = []
        for ins in bb.instructions:
            si = getattr(ins, "sync_info", None)
            ow = list(si.on_wait) if si is not None and si.on_wait else []
            if len(ow) > 1:
                for w in ow[:-1]:
                    n_new += 1
                    nop = mybir.InstNoOp(
                        name=f"{ins.name}_sw{n_new}",
                        engine=ins.engine,
                        sync_info=mybir.SyncInfo(on_wait=[w], on_update=[]),
                    )
                    new.append(nop)
                ins.sync_info = mybir.SyncInfo(
                    on_wait=[ow[-1]],
                    on_update=list(si.on_update) if si.on_update else [],
                )
            new.append(ins)
        bb.instructions = new


_NC = None
_LAST = None


def _ensure_ntff_hook():
    """The agent image's antenv lacks axon_hooks; shim it and register the
    ctypes NTFF profiler from trn_boot so trace=True yields exec_time_ns."""
    import sys
    import types
    try:
        import antenv.axon_hooks  # noqa: F401
        return
    except ImportError:
        pass
    mod = types.ModuleType("antenv.axon_hooks")
    holder = [None]
    mod.set_axon_ntff_profile_hook = lambda h: holder.__setitem__(0, h)
    mod.get_axon_ntff_profile_hook = lambda: holder[0]
    sys.modules["antenv.axon_hooks"] = mod
    import antenv
    antenv.axon_hooks = mod
    try:
        sys.path.insert(0, "/root/.axon_site")
        from trn_agent_boot.trn_boot import _ntff_profile_via_ctypes
        mod.set_axon_ntff_profile_hook(
            _ntff_profile_via_ctypes("/opt/axon/libaxon_pjrt.so"))
    except Exception:
        pass


def kernel(**inputs):
    global _NC, _LAST
    bf = ml_dtypes.bfloat16
    x = np.asarray(inputs["x"], np.float32)
    qkv_w = np.asarray(inputs["qkv_w"], np.float32)
    proj_w = np.asarray(inputs["proj_w"], np.float32)
    proj_b = np.asarray(inputs["proj_b"], np.float32)
    a1 = np.asarray(inputs["lora_w1_l1"], np.float32)
    b1 = np.asarray(inputs["lora_w1_l2"], np.float32)
    a2 = np.asarray(inputs["lora_w2_l1"], np.float32)
    b2 = np.asarray(inputs["lora_w2_l2"], np.float32)

    # fold LoRA into the dense weights (exact: x@W.T + (x@A.T)@B.T*2 = x@(W+2BA).T)
    Wqkv = qkv_w + 2.0 * (b1 @ a1)
    Wp = proj_w + 2.0 * (b2 @ a2)

    xTg = [np.ascontiguousarray(x[g].T).astype(bf) for g in range(B)]
    in_maps = []
    for c in range(8):
        g, hg = divmod(c, 4)
        r0 = hg * 256
        Kg = Wqkv[1024 + r0:1024 + r0 + 256]        # [256, 1024]
        Qg = Wqkv[r0:r0 + 256]
        Vg = Wqkv[2048 + r0:2048 + r0 + 256]
        m = {
            "xT": xTg[g],
            "wqkT": np.ascontiguousarray(np.vstack([Kg, Qg]).T).astype(bf),
            "wvT": np.ascontiguousarray(Vg.T).astype(bf),
            "projT": np.ascontiguousarray(Wp[:, r0:r0 + 256].T).astype(bf),
        }
        in_maps.append(m)

    if _NC is None:
        _NC = build()
    trace = os.environ.get("ATT_TRACE", "0") == "1"
    if trace:
        _ensure_ntff_hook()
    _LAST = run_bass_kernel_spmd(_NC, in_maps, core_ids=list(range(8)),
                                 trace=trace)
    out = np.empty((B, N, C), np.float32)
    for g in range(B):
        acc = np.zeros((C, N), np.float32)
        for hg in range(4):
            acc += np.asarray(_LAST.results[4 * g + hg]["outT"], np.float32)
        out[g] = acc.T + proj_b[None, :]
    return out
